# revision 19
# baseline (speedup 1.0000x reference)
"""GaussianImage rasterization kernel for Trainium2 (8 NeuronCores).

Math: out(h,w,c) = rgb[-1,c]*alpha[-1] * S(h,w),
      S = sum_n exp(-0.5 (p-m_n)^T InvCov_n (p-m_n))

Fast path (tensor-product pixel grid, which setup_inputs' meshgrid is):
each gaussian is factorized over the grid with Mehler's formula
    exp(-(u^2 - 2 rho u v + v^2)/(2(1-rho^2)))
      = sqrt(1-rho^2) * sum_j (rho^j/j!) He_j(u) He_j(v) e^{-u^2/2} e^{-v^2/2}
so S restricted to a core's (128h x 256w) tile is a single K-row matmul
S = Q^T P with host-precomputed fp16 factor rows (no device exp at all).
Rank is chosen per (gaussian, core) empirically; gaussians whose |rho| is
too close to 1 (rank > J_CAP) go through a direct path instead: within an
image column w, x is constant, so expo is a quadratic in y -> one matmul
(K=12 fp16-split rows, N = 256 w * NSL slots) -> ScalarE exp -> per-w
slot-reduce.  Per-(gaussian, column) support culling keeps NSL small.

Work per core: PE ~6.4k cycles, ACT exp only ~NSL*256 elements (vs 4.2M
for the dense baseline), DMA ~1MB in / 393KB out.

Fallback path (non-tensor-product pixels): dense feature matmul + exp
over all (pixel, gaussian) pairs — the previous 56us baseline.
"""

import math

import numpy as np

N_GAUSS = 128
H = 512
W = 512
N_CORES = 8

# --- fast-path tuning ---
TAU = 2e-3        # per-(gaussian, core) Mehler truncation tolerance
J_CAP = 96        # ranks above this -> direct path
DCUT = -7.0       # cull direct (gaussian, column) pairs with max expo below
NSL_MAX = 11      # max direct slots per round (PSUM budget)
PAD_C = -240.0    # padding const-coef (x256 => expo -61440 -> exp = 0)

_CACHE = {}


def _f16(a):
    return np.asarray(a, np.float64).astype(np.float16).astype(np.float64)


def _split2(a):
    hi = np.asarray(a, np.float64).astype(np.float16).astype(np.float64)
    lo = (a - hi).astype(np.float16).astype(np.float64)
    return hi, lo


def _split3(a):
    hi = np.asarray(a, np.float64).astype(np.float16).astype(np.float64)
    r = a - hi
    mid = r.astype(np.float16).astype(np.float64)
    lo = (r - mid).astype(np.float16).astype(np.float64)
    return hi, mid, lo


def _quad_coeffs(mean, scale, theta):
    """Per-gaussian inverse-covariance quadratic: expo =
    -0.5*(A xt^2 + 2B xt yt + C yt^2), xt = x-mx, yt = y-my."""
    m = mean.astype(np.float64)
    s = scale.astype(np.float64)
    th = (1.0 + np.sin(theta.astype(np.float64)[:, 0])) * np.pi
    c, sn = np.cos(th), np.sin(th)
    with np.errstate(divide='ignore', invalid='ignore'):
        is1 = 1.0 / s[:, 0] ** 2
        is2 = 1.0 / s[:, 1] ** 2
        A = c * c * is1 + sn * sn * is2
        B = c * sn * (is1 - is2)
        C = sn * sn * is1 + c * c * is2
        det = A * C - B * B
        rho = -B / np.sqrt(A * C)
        sigx = np.sqrt(C / det)
        sigy = np.sqrt(A / det)
    return m, A, B, C, rho, sigx, sigy


def _mehler_rows(u, v, r, sgn, tau, jcap):
    """Factor rows of exp(-(u^2-2r' u v+v^2)/(2(1-r'^2))) on grids u, v.
    Returns (p_rows, q_rows) lists or None if rank exceeds jcap."""
    r = min(max(r, 1e-12), 1.0 - 1e-12)
    pref = (1.0 - r * r) ** 0.25
    pu = pref * np.exp(-u * u / 2)
    qv = pref * np.exp(-v * v / 2)
    prev_u = prev_v = None
    p_rows, q_rows = [], []
    j = 0
    while True:
        if np.abs(pu).max() * np.abs(qv).max() / max(1.0 - r, 1e-6) < tau:
            return p_rows, q_rows
        if j >= jcap:
            return None
        p_rows.append(pu if (sgn > 0 or j % 2 == 0) else -pu)
        q_rows.append(qv)
        cu = math.sqrt(r / (j + 1))
        cp = r * math.sqrt(j / (j + 1)) if j else 0.0
        nu = cu * u * pu - (cp * prev_u if prev_u is not None else 0.0)
        nv = cu * v * qv - (cp * prev_v if prev_v is not None else 0.0)
        prev_u, pu = pu, nu
        prev_v, qv = qv, nv
        j += 1


def _prep_fast(mean, rgb, alpha, scale, theta, pixels):
    """Build per-core operands. Returns (shape_key, in_maps)."""
    X = np.asarray(pixels[0, :, 0], np.float64)
    Y = np.asarray(pixels[:, 0, 1], np.float64)
    m, A, B, C, rho, sigx, sigy = _quad_coeffs(mean, scale, theta)
    ok = np.isfinite(A) & np.isfinite(B) & np.isfinite(C) & np.isfinite(rho)

    cores = []
    max_k = 1
    max_slots = 1
    for core in range(N_CORES):
        hb, wb = core % 4, core // 4
        Xc = X[wb * 256:(wb + 1) * 256]
        Yc = Y[hb * 128:(hb + 1) * 128]
        p_rows, q_rows = [], []
        direct = []
        for n in range(N_GAUSS):
            if not ok[n]:
                continue
            u = (Xc - m[n, 0]) / sigx[n]
            v = (Yc - m[n, 1]) / sigy[n]
            sgn = 1.0 if rho[n] >= 0 else -1.0
            res = _mehler_rows(u, v, abs(rho[n]), sgn, TAU, J_CAP)
            if res is None:
                direct.append(n)
            else:
                p_rows += res[0]
                q_rows += res[1]
        # direct path: per-column quadratic in yt = y-0.5
        yt = Yc - 0.5
        acol = []   # per active (n,w): (w, a, b, c)
        slot_cnt = np.zeros(256, np.int64)
        slot_of = []
        for n in direct:
            dy0 = 0.5 - m[n, 1]
            xt = Xc - m[n, 0]
            a = np.full(256, -0.5 * C[n])
            b = -(B[n] * xt + C[n] * dy0)
            cc = -0.5 * (A[n] * xt * xt + 2 * B[n] * xt * dy0 + C[n] * dy0 * dy0)
            vx = -b / (2 * a)
            mx_ = np.where(np.abs(vx) <= 0.5, cc - b * b / (4 * a),
                           np.maximum(a * 0.25 + b * 0.5 + cc,
                                      a * 0.25 - b * 0.5 + cc))
            for w in np.nonzero(mx_ > DCUT)[0]:
                acol.append((w, slot_cnt[w], a[w], b[w], cc[w]))
                slot_cnt[w] += 1
        cores.append((p_rows, q_rows, yt, acol))
        max_k = max(max_k, len(p_rows))
        max_slots = max(max_slots, int(slot_cnt.max()) if len(acol) else 0)

    KCH = (max_k + 127) // 128
    max_slots = max(max_slots, 1)
    NSL = min(max_slots, NSL_MAX)
    R = (max_slots + NSL - 1) // NSL
    SL = R * NSL

    rgba = (np.asarray(rgb[-1], np.float64) * np.asarray(alpha[-1], np.float64)[0])
    rgba_b = np.zeros((128, 4), dtype=np.float32)
    rgba_b[:, :3] = rgba.astype(np.float32)[None, :]

    ident = np.eye(128, dtype=np.float16)
    rgba16 = rgba_b[0:1, :].view(np.float16)         # (1, 8)

    in_maps = []
    for core in range(N_CORES):
        p_rows, q_rows, yt, acol = cores[core]
        K = len(p_rows)
        qm = np.zeros((128, KCH * 128), np.float16)
        pm = np.zeros((128, KCH * 256), np.float16)
        for k in range(K):
            c, p = k // 128, k % 128
            qm[p, c * 128:(c + 1) * 128] = q_rows[k].astype(np.float16)
            pm[p, c * 256:(c + 1) * 256] = p_rows[k].astype(np.float16)
        # qmx = qm | identity | rgba bits (replicated to 128 partitions)
        qmx = np.concatenate(
            [qm, ident, np.broadcast_to(rgba16, (128, 8))], axis=1)
        # direct features (fp16 splits, power-of-2 scaled)
        y2h, y2l = _split2(1024.0 * yt * yt)
        yh, yl = _split2(512.0 * yt)
        one = np.full(128, 256.0)
        zero = np.zeros(128)
        fy = np.stack([y2h, y2l, y2h, y2h, yh, yl, yh, yh, one, one, one,
                       zero]).astype(np.float16)
        av = np.zeros((256, SL)); bv = np.zeros((256, SL))
        cv = np.full((256, SL), PAD_C * 256.0)
        for (w, s, a, b, cc) in acol:
            av[w, s] = a; bv[w, s] = b; cv[w, s] = cc
        ah, am, al = _split3(av / 1024.0)
        bh, bm, bl = _split3(bv / 512.0)
        ch, cm, cl = _split3(cv / 256.0)
        zz = np.zeros_like(ah)
        # row pairing vs fy: [(y2h,ah),(y2l,ah),(y2h,am),(y2h,al),
        #                     (yh,bh),(yl,bh),(yh,bm),(yh,bl),(1,ch),(1,cm),(1,cl)]
        g = np.stack([ah, ah, am, al, bh, bh, bm, bl, ch, cm, cl, zz])
        # slot-major within each w-half: round r, half hf block has
        # col = s*128 + w  (contiguous id-matmul rhs slices)
        gdh = [np.zeros((12, R * NSL * 128), np.float16) for _ in (0, 1)]
        for r in range(R):
            for hf in (0, 1):
                blk = g[:, hf * 128:(hf + 1) * 128, r * NSL:(r + 1) * NSL]
                blk = blk.transpose(0, 2, 1)            # (12, NSL, 128w)
                gdh[hf][:, r * NSL * 128:(r + 1) * NSL * 128] = \
                    blk.reshape(12, NSL * 128).astype(np.float16)
        in_maps.append({
            "fga1": np.ascontiguousarray(
                np.concatenate([fy, gdh[0][:, 0:512]], axis=1)),
            "fga2": np.ascontiguousarray(gdh[0][:, 512:]),
            "fgb1": np.ascontiguousarray(gdh[1][:, 0:512]),
            "fgb2": np.ascontiguousarray(gdh[1][:, 512:]),
            "qmx": np.ascontiguousarray(qmx),
            "pm": np.ascontiguousarray(pm),
        })
    return (KCH, R, NSL), in_maps


# ---------------------------------------------------------------------------
# Fast-path device kernel
# ---------------------------------------------------------------------------

def _build_fast(KCH, R, NSL):
    import concourse.bacc as bacc
    import concourse.mybir as mybir
    from concourse.tile import TileContext

    fp16 = mybir.dt.float16
    f32 = mybir.dt.float32
    Exp = mybir.ActivationFunctionType.Exp

    HCOL = NSL * 128            # direct cols per round per w-half
    PSD = (HCOL * 4 + 2047) // 2048 * 512   # bank-aligned psum cols
    HC2 = R * HCOL - 512        # tail beyond the first 512 cols
    nc = bacc.Bacc("TRN2", target_bir_lowering=False)
    fga1_d = nc.dram_tensor("fga1", [12, 640], fp16, kind="ExternalInput")
    fga2_d = nc.dram_tensor("fga2", [12, HC2], fp16, kind="ExternalInput")
    fgb1_d = nc.dram_tensor("fgb1", [12, 512], fp16, kind="ExternalInput")
    fgb2_d = nc.dram_tensor("fgb2", [12, HC2], fp16, kind="ExternalInput")
    qmx_d = nc.dram_tensor("qmx", [128, KCH * 128 + 136], fp16,
                           kind="ExternalInput")
    pm_d = nc.dram_tensor("pm", [128, KCH * 256], fp16,
                          kind="ExternalInput")
    out_d = nc.dram_tensor("out", [128, 768], f32, kind="ExternalOutput")

    with TileContext(nc) as tc:
        with (
            tc.tile_pool(name="const", bufs=1) as cpool,
            tc.tile_pool(name="mrows", bufs=1) as mpool,
            tc.tile_pool(name="ed", bufs=1) as epool,
            tc.tile_pool(name="acc", bufs=1) as apool,
            tc.tile_pool(name="psd", bufs=1, space="PSUM") as pdpool,
            tc.tile_pool(name="psm", bufs=1, space="PSUM") as pmpool,
        ):
            # warm the exp table immediately (ACT busy ~2.7us); all DMAs
            # and matmuls overlap under it.
            dummy = cpool.tile([128, 1], fp16, tag="dummy")
            nc.vector.memset(dummy[:], 0)
            nc.scalar.activation(dummy[:], dummy[:], Exp)

            # direct-path operands first on BOTH hwdge rings (ring FIFO =>
            # the big mehler operands behind them can't contend)
            fga1_sb = cpool.tile([12, 640], fp16, tag="fga1")
            fga2_sb = cpool.tile([12, HC2], fp16, tag="fga2")
            fgb1_sb = cpool.tile([12, 512], fp16, tag="fgb1")
            fgb2_sb = cpool.tile([12, HC2], fp16, tag="fgb2")
            nc.sync.dma_start(fga1_sb[:], fga1_d[:])
            nc.scalar.dma_start(fga2_sb[:], fga2_d[:])
            nc.sync.dma_start(fgb1_sb[:], fgb1_d[:])
            nc.scalar.dma_start(fgb2_sb[:], fgb2_d[:])
            fy = fga1_sb[:, 0:128]

            # warmup tile: keep the PE HAM busy so real matmuls run at
            # 2.4GHz instead of the cold 1.2GHz
            wm = cpool.tile([128, 128], fp16, tag="wm")
            nc.vector.memset(wm[:], 0)

            qmx_sb = mpool.tile([128, KCH * 128 + 136], fp16, tag="qmx")
            nc.sync.dma_start(qmx_sb[:], qmx_d[:])
            ident = qmx_sb[:, KCH * 128:KCH * 128 + 128]
            rgba = qmx_sb[:, KCH * 128 + 128:KCH * 128 + 136].bitcast(f32)

            pm_sb = mpool.tile([128, KCH * 256], fp16, tag="pm")
            nc.scalar.dma_start(pm_sb[:], pm_d[:])

            # ---- direct-path expo matmuls (per w-half tiles) ----
            ps_d = [pdpool.tile([128, PSD], f32, tag=f"psd{h}",
                                name=f"psd{h}") for h in (0, 1)]
            ed = [epool.tile([128, R * HCOL], fp16, tag=f"ed{h}",
                             name=f"ed{h}") for h in (0, 1)]
            for i in range(26):
                nc.tensor.matmul(ps_d[0][:, 0:128], wm[:], wm[:])
            for hf in (0, 1):
                t1, t2 = (fga1_sb, fga2_sb) if hf == 0 else (fgb1_sb, fgb2_sb)
                off1 = 128 if hf == 0 else 0
                nc.tensor.matmul(ps_d[hf][:, 0:512], fy,
                                 t1[:, off1:off1 + 512])
                for lo in range(0, HC2, 512):
                    hi = min(lo + 512, HC2)
                    nc.tensor.matmul(ps_d[hf][:, 512 + lo:512 + hi], fy,
                                     t2[:, lo:hi])
            # exp: PSUM -> SBUF fp16 (one call per half per round)
            for r in range(R):
                for hf in (0, 1):
                    nc.scalar.activation(
                        ed[hf][:, r * HCOL:(r + 1) * HCOL],
                        ps_d[hf][:, 0:HCOL], Exp)

            # ---- one accumulation group: Mehler chunks (N=256) +
            # identity slot-reduction matmuls (slot-major rhs) ----
            ps_m = pmpool.tile([128, 256], f32, tag="psm")
            out_big = apool.tile([128, 768], f32, tag="outbig")
            ob3 = out_big[:].rearrange("p (c w) -> p c w", c=3)
            for c in range(KCH):
                nc.tensor.matmul(
                    ps_m[:], qmx_sb[:, c * 128:(c + 1) * 128],
                    pm_sb[:, c * 256:(c + 1) * 256],
                    start=(c == 0), stop=False)
            for hf in (0, 1):
                dst = ps_m[:, hf * 128:(hf + 1) * 128]
                for r in range(R):
                    eh = ed[hf][:, r * HCOL:(r + 1) * HCOL]
                    for s in range(NSL):
                        nc.tensor.matmul(
                            dst, ident, eh[:, s * 128:(s + 1) * 128],
                            start=False,
                            stop=(hf == 1 and r == R - 1 and s == NSL - 1))
            # ---- channel scale (contiguous planar writes) + store ----
            nc.vector.tensor_scalar_mul(ob3[:, 0], ps_m[:], rgba[:, 0:1])
            nc.vector.tensor_scalar_mul(ob3[:, 1], ps_m[:], rgba[:, 1:2])
            nc.scalar.mul(ob3[:, 2], ps_m[:], rgba[:, 2:3])
            nc.sync.dma_start(out_d[:], out_big[:])

    nc.finalize()
    return nc


def _run_fast(inputs, trace=False):
    from concourse.bass_utils import run_bass_kernel_spmd

    key, in_maps = _prep_fast(**inputs)
    ck = ("fast",) + key
    if ck not in _CACHE:
        _CACHE[ck] = _build_fast(*key)
    nc = _CACHE[ck]
    res = run_bass_kernel_spmd(
        nc, in_maps, core_ids=list(range(N_CORES)), trace=trace,
    )
    full = np.zeros((H, W, 3), np.float32)
    for core in range(N_CORES):
        hb, wb = core % 4, core // 4
        o = np.asarray(res.results[core]["out"]).reshape(128, 3, 256)
        o = o.transpose(0, 2, 1)
        full[hb * 128:(hb + 1) * 128, wb * 256:(wb + 1) * 256] = o
    return full, res


# ---------------------------------------------------------------------------
# Fallback path (arbitrary pixel grids): dense feature matmul + exp
# ---------------------------------------------------------------------------

ROWS_PER_CORE = H // N_CORES          # 64
PX_PER_CORE = ROWS_PER_CORE * W       # 32768
N_BLOCKS = PX_PER_CORE // 128         # 256 blocks of 128 px
N_ROUNDS = N_BLOCKS // 16             # 16 rounds x 16 blocks

FEAT_CHUNKS = [1024, 1024, 2048, 2048, 2048, 4096, 4096, 8192, 8192]
FEAT_OFFS = [0]
for _w in FEAT_CHUNKS:
    FEAT_OFFS.append(FEAT_OFFS[-1] + _w)


def _fb_coeffs(mean, scale, theta):
    m = mean.astype(np.float64)
    s = scale.astype(np.float64)
    th = (1.0 + np.sin(theta.astype(np.float64)[:, 0])) * np.pi
    c, sn = np.cos(th), np.sin(th)
    is1 = 1.0 / s[:, 0] ** 2
    is2 = 1.0 / s[:, 1] ** 2
    A = c * c * is1 + sn * sn * is2
    B = c * sn * (is1 - is2)
    C = sn * sn * is1 + c * c * is2
    mx = m[:, 0] - 0.5
    my = m[:, 1] - 0.5
    g = np.stack([
        -A / 8.0,
        -B / 4.0,
        -C / 8.0,
        (A * mx + B * my) / 2.0,
        (B * mx + C * my) / 2.0,
        -0.5 * (A * mx * mx + 2.0 * B * mx * my + C * my * my),
    ], axis=0)
    return g


def _fb_features(pixels_flat):
    p = pixels_flat.astype(np.float64)
    x = p[:, 0] - 0.5
    y = p[:, 1] - 0.5
    return np.stack([4*x*x, 4*x*y, 4*y*y, 2*x, 2*y, np.ones_like(x)], axis=0)


def _fb_row_plan():
    plan = []
    big = [2, 1, 4, 0, 3]
    for f in big:
        plan.append((f, 0, 0))
    plan.append((5, 0, 0))
    plan.append((5, 0, 1))
    for f in big:
        plan.append((f, 0, 1))
        plan.append((f, 1, 0))
    for f in big:
        plan.append((f, 1, 1))
        plan.append((f, 0, 2))
        plan.append((f, 2, 0))
    return plan


def _fb_host_prep(mean, rgb, alpha, scale, theta, pixels):
    plan = _fb_row_plan()
    g = _fb_coeffs(mean, scale, theta)
    g_pieces = [_split3(g[f]) for f in range(6)]
    coef = np.stack([g_pieces[f][gp] for (f, _fp, gp) in plan],
                    axis=0).astype(np.float16)
    rgba = (rgb[-1].astype(np.float64) * alpha[-1, 0].astype(np.float64))
    rgba_b = np.zeros((128, 4), dtype=np.float32)
    rgba_b[:, :3] = rgba.astype(np.float32)[None, :]
    pix = np.asarray(pixels).reshape(H * W, 2)
    feats = []
    for core in range(N_CORES):
        pf = pix[core * PX_PER_CORE:(core + 1) * PX_PER_CORE]
        F = _fb_features(pf)
        f_pieces = [_split3(F[f]) for f in range(6)]
        F32 = np.stack([f_pieces[f][fp] for (f, fp, _gp) in plan], axis=0)
        Fb = F32.reshape(32, 128, 256)
        Fb = Fb.transpose(0, 2, 1)
        Fsb = Fb.reshape(32, 256 * 128)
        feats.append(np.ascontiguousarray(Fsb.astype(np.float16)))
    return feats, coef, rgba_b


def _build_fallback():
    import concourse.bacc as bacc
    import concourse.mybir as mybir
    from concourse.tile import TileContext

    fp16 = mybir.dt.float16
    f32 = mybir.dt.float32

    nc = bacc.Bacc("TRN2", target_bir_lowering=False)
    feat_d = [
        nc.dram_tensor(f"feat{t}", [32, w], fp16, kind="ExternalInput")
        for t, w in enumerate(FEAT_CHUNKS)
    ]
    coef_d = nc.dram_tensor("coef", [32, 128], fp16, kind="ExternalInput")
    rgba_d = nc.dram_tensor("rgba", [128, 4], f32, kind="ExternalInput")
    out_d = nc.dram_tensor("out", [128, 768], f32, kind="ExternalOutput")

    with TileContext(nc) as tc:
        with (
            tc.tile_pool(name="const", bufs=1) as cpool,
            tc.tile_pool(name="feat", bufs=1) as fpool,
            tc.tile_pool(name="psum", bufs=2, space="PSUM") as ppool,
            tc.tile_pool(name="splat", bufs=2) as spool,
            tc.tile_pool(name="scratch", bufs=2) as scpool,
            tc.tile_pool(name="acc", bufs=1) as apool,
        ):
            dummy = cpool.tile([128, 1], fp16, tag="dummy")
            nc.gpsimd.memset(dummy[:], 0)
            nc.scalar.activation(dummy[:], dummy[:],
                                 mybir.ActivationFunctionType.Exp)

            g_sb = cpool.tile([32, 128], fp16, tag="gsb")
            nc.scalar.dma_start(g_sb[:], coef_d[:])
            rgba_sb = cpool.tile([128, 4], f32, tag="rgba")
            nc.scalar.dma_start(rgba_sb[:], rgba_d[:])

            ftiles = []
            for t, fd in enumerate(feat_d):
                ft = fpool.tile(list(fd.shape), fp16, tag=f"ft{t}")
                nc.sync.dma_start(ft[:], fd[:])
                ftiles.append(ft)

            S_big = apool.tile([128, 256], f32, tag="sbig")
            out_big = apool.tile([128, 768], f32, tag="outbig")

            for r in range(N_ROUNDS):
                ps = ppool.tile([128, 2048], f32, tag="ps")
                for i in range(16):
                    g = r * 2048 + i * 128
                    t = next(c for c in range(len(FEAT_CHUNKS))
                             if FEAT_OFFS[c + 1] > g)
                    off = g - FEAT_OFFS[t]
                    nc.tensor.matmul(
                        ps[:, i * 128:(i + 1) * 128],
                        ftiles[t][:, off:off + 128], g_sb[:],
                    )
                sp = spool.tile([128, 2048], fp16, tag="sp")
                nc.scalar.activation(sp[:], ps[:],
                                     mybir.ActivationFunctionType.Exp)
                sp3 = sp[:].rearrange("p (i g) -> p i g", g=128)
                sc = scpool.tile([128, 1024], fp16, tag="sc")
                sc3 = sc[:].rearrange("p (i g) -> p i g", g=64)
                eng = nc.vector if r % 2 == 0 else nc.gpsimd
                eng.tensor_tensor(
                    sc3, sp3[:, :, 0:64], sp3[:, :, 64:128],
                    op=mybir.AluOpType.add,
                )
                nc.vector.tensor_reduce(
                    S_big[:, 16 * r:16 * (r + 1)], sc3,
                    axis=mybir.AxisListType.X, op=mybir.AluOpType.add,
                )

                if r == 7 or r == 15:
                    h = 0 if r == 7 else 1
                    ob3 = out_big[:].rearrange("p (j c) -> p j c", c=3)
                    for c in range(3):
                        nc.scalar.activation(
                            ob3[:, 128 * h:128 * (h + 1), c],
                            S_big[:, 128 * h:128 * (h + 1)],
                            mybir.ActivationFunctionType.Copy,
                            scale=rgba_sb[:, c:c + 1],
                        )
                    nc.sync.dma_start(out_d[:, 384 * h:384 * (h + 1)],
                                      out_big[:, 384 * h:384 * (h + 1)])

    nc.finalize()
    return nc


def _run_fallback(inputs, trace=False):
    from concourse.bass_utils import run_bass_kernel_spmd

    feats, coef, rgba_b = _fb_host_prep(**inputs)
    if "fallback" not in _CACHE:
        _CACHE["fallback"] = _build_fallback()
    nc = _CACHE["fallback"]

    in_maps = []
    for core in range(N_CORES):
        fc = feats[core]
        mmap = {f"feat{t}": np.ascontiguousarray(
                    fc[:, FEAT_OFFS[t]:FEAT_OFFS[t + 1]])
                for t in range(len(FEAT_CHUNKS))}
        mmap["coef"] = coef
        mmap["rgba"] = rgba_b
        in_maps.append(mmap)

    res = run_bass_kernel_spmd(
        nc, in_maps, core_ids=list(range(N_CORES)), trace=trace,
    )
    shards = []
    for core in range(N_CORES):
        o = np.asarray(res.results[core]["out"]).reshape(128, 256, 3)
        o = o.reshape(64, 2, 256, 3)
        shards.append(o.reshape(64, 512, 3))
    full = np.concatenate(shards, axis=0).astype(np.float32)
    return full, res


# ---------------------------------------------------------------------------

def _is_tensor_product(pixels):
    p = np.asarray(pixels)
    if p.shape != (H, W, 2):
        return False
    return (np.abs(p[:, :, 0] - p[0:1, :, 0]).max() == 0.0 and
            np.abs(p[:, :, 1] - p[:, 0:1, 1]).max() == 0.0)


def _run(inputs, trace=False):
    inputs = {k: np.asarray(v) for k, v in inputs.items()}
    if _is_tensor_product(inputs["pixels"]):
        return _run_fast(inputs, trace=trace)
    return _run_fallback(inputs, trace=trace)


def kernel(mean, rgb, alpha, scale, theta, pixels):
    out, _ = _run(dict(mean=mean, rgb=rgb, alpha=alpha, scale=scale,
                       theta=theta, pixels=pixels))
    return out
